# revision 6
# baseline (speedup 1.0000x reference)
"""Trainium2 Bass kernel for suffix-softmax attention visualization.

Computes, for hidden_states [S, B, H], W [H, 1], b [1]:
    s[t, b]   = sum_h hidden_states[t, b, h] * W[h, 0] + b[0]
    out[t, b] = exp(s[t, b]) / sum_{t' >= t} exp(s[t', b])     (suffix softmax)
returned as [S, B, 1] f32.  The softmax ratio is shift-invariant so b cancels.

Sharding: data-parallel over batch — 8 NeuronCores, 8 batch columns each.

v3 design (memory regime; per-core floor = 64 MiB f32 HBM reads):
  - hs streams via SWDGE (gpsimd) DMAs with an inline f32->fp16 cast: HBM
    reads stay f32 (unavoidable) but SBUF tiles are fp16, which unlocks the
    DVE 2x_1p perf mode for the multiply.  Middle blocks ride 4 MiB
    two-block DMAs (fewer completions -> higher sustained rate); the first
    and last seq blocks are split into 4-column pieces so compute ramps
    while the stream warms up and the tail chain after the last byte is
    short.
  - One DVE tensor_tensor multiply per DMA unit (fp16, 2x) forms
    prod = hs * w for all its columns in a single instruction.
  - The h-reduction is split per block between ScalarE activation
    (Copy, accum_out=...) and one grouped VectorE tensor_reduce.
  - Blocks are processed in REVERSE seq order with a running esum[p, b] =
    sum of e over already-processed (later) blocks.  Per block the suffix
    denominator = tri @ e_blk + ones @ esum lands in PSUM via two tiny PE
    matmuls, so there is no global scan epilogue; out chunks DMA back
    while earlier blocks still stream in.
  - Output is written [128 p, 32 j, 8 b] (j = reversed block index,
    contiguous per partition); the host unscrambles to [4096, 8].
"""

import numpy as np

import concourse.bacc as bacc
import concourse.mybir as mybir
import concourse.tile as tile
from concourse import bass_utils

P = 128
S = 4096
B = 64
H = 512
N_CORES = 8
BC = B // N_CORES  # batch columns per core
NBLK = S // P      # 32 seq blocks per core
C = NBLK * BC      # output columns per partition, c = j*BC + b

F32 = mybir.dt.float32
F16 = mybir.dt.float16

Copy = mybir.ActivationFunctionType.Copy
Exp = mybir.ActivationFunctionType.Exp
X = mybir.AxisListType.X


def build_program():
    """Per-core program.

    Inputs : hs [S, BC, H] f32, wrep [P, BC*H] fp16 (w tiled BC times),
             tri [P, P] f32 lower-triangular ones, onesq [P, P] f32 ones.
    Output : out [P, C] f32, out[p, j*BC+b] = selected[(NBLK-1-j)*P + p, b].
    """
    nc = bacc.Bacc("TRN2", target_bir_lowering=False, debug=False)
    hs = nc.dram_tensor("hs", [S, BC, H], F32, kind="ExternalInput")
    wrep = nc.dram_tensor("wrep", [P, BC * H], F16, kind="ExternalInput")
    tri = nc.dram_tensor("tri", [P, P], F32, kind="ExternalInput")
    onesq = nc.dram_tensor("onesq", [P, P], F32, kind="ExternalInput")
    out = nc.dram_tensor("out", [P, C], F32, kind="ExternalOutput")

    with tile.TileContext(nc) as tc:
        with (
            tc.tile_pool(name="hsp", bufs=5) as hsp,      # [P,2,BC,H] f16 pairs
            tc.tile_pool(name="hs1p", bufs=2) as hs1p,    # single blocks
            tc.tile_pool(name="prodp", bufs=3) as prodp,
            tc.tile_pool(name="consts", bufs=1) as consts,
            tc.tile_pool(name="scp", bufs=4) as scp,
            tc.tile_pool(name="ep", bufs=6) as ep,
            tc.tile_pool(name="work", bufs=1) as work,
            tc.tile_pool(name="psum", bufs=6, space="PSUM") as psum,
        ):
            # Constants ride the ACT HWDGE ring so they don't contend with
            # the gpsimd stream queue.
            wrep_t = consts.tile([P, BC * H], F16)
            nc.scalar.dma_start(out=wrep_t, in_=wrep.ap())
            tri_t = consts.tile([P, P], F32)
            nc.scalar.dma_start(out=tri_t, in_=tri.ap())
            onesq_t = consts.tile([P, P], F32)
            nc.scalar.dma_start(out=onesq_t, in_=onesq.ap())

            wrep_v = wrep_t.rearrange("p (b h) -> p b h", h=H)

            # ---- stream DMAs, reverse seq order --------------------------
            # processing order j=0..31 maps to seq block k=31-j.
            # j=0: two 4-col piece DMAs; j=1: single; pairs (2,3)..(28,29);
            # j=30: single; j=31: two 4-col piece DMAs.
            hs_ap = hs.ap()

            def blk_rows(j):
                k = NBLK - 1 - j
                return hs_ap[k * P : (k + 1) * P, :, :]

            dmas = {}  # j -> tile view [P, BC, H] (fp16)
            t0 = hs1p.tile([P, BC, H], F16, name="hs_single")
            nc.gpsimd.dma_start(out=t0[:, : BC // 2, :], in_=blk_rows(0)[:, : BC // 2, :])
            nc.gpsimd.dma_start(out=t0[:, BC // 2 :, :], in_=blk_rows(0)[:, BC // 2 :, :])
            dmas[0] = t0
            t1 = hs1p.tile([P, BC, H], F16, name="hs_single")
            nc.gpsimd.dma_start(out=t1, in_=blk_rows(1))
            dmas[1] = t1
            for jp in range(2, NBLK - 2, 2):
                # pair covers j=jp (seq k) and j=jp+1 (seq k-1); DRAM rows
                # [k-1 .. k+1) reordered so two=0 -> seq k, two=1 -> seq k-1.
                k = NBLK - 1 - jp
                rows = hs_ap[(k - 1) * P : (k + 1) * P, :, :]
                pair = hsp.tile([P, 2, BC, H], F16, name="hs_pair")
                pr = rows.rearrange("(two p) b h -> p two b h", p=P)
                # two=0 is DRAM rows (k-1)*P.. (seq block k-1 = j=jp+1)
                nc.gpsimd.dma_start(out=pair, in_=pr)
                dmas[jp] = pair[:, 1]      # seq k   (earlier j)
                dmas[jp + 1] = pair[:, 0]  # seq k-1
                dmas[(jp, "pair")] = pair
            t30 = hs1p.tile([P, BC, H], F16, name="hs_single")
            nc.gpsimd.dma_start(out=t30, in_=blk_rows(30))
            dmas[30] = t30
            t31 = hs1p.tile([P, BC, H], F16, name="hs_single")
            nc.gpsimd.dma_start(out=t31[:, : BC // 2, :], in_=blk_rows(31)[:, : BC // 2, :])
            nc.gpsimd.dma_start(out=t31[:, BC // 2 :, :], in_=blk_rows(31)[:, BC // 2 :, :])
            dmas[31] = t31

            dummy = work.tile([P, 1], F32)
            esum = [
                work.tile([P, BC], F32, name="esum0"),
                work.tile([P, BC], F32, name="esum1"),
            ]
            nc.vector.memset(esum[0], 0.0)
            sel = work.tile([P, C], F32)
            out_ap = out.ap()

            def reduce_cols(prod_v, s_col, lo, hi, act_cols):
                """s_col[:, lo:hi] = sum_h prod_v[:, lo:hi, :], ACT cols first."""
                for b in range(lo, lo + act_cols):
                    nc.scalar.activation(
                        dummy.broadcast_to((P, H)),
                        prod_v[:, b, :],
                        Copy,
                        accum_out=s_col[:, b : b + 1],
                    )
                if lo + act_cols < hi:
                    nc.vector.reduce_sum(
                        out=s_col[:, lo + act_cols : hi],
                        in_=prod_v[:, lo + act_cols : hi, :],
                        axis=X,
                    )

            def scan_block(j, e_t, ncols, col0=0, last=False):
                """Suffix-scan update for ncols columns [col0, col0+ncols) of
                processed block j: denominators into PSUM, esum update, and
                selected values into sel."""
                lo, hi = col0, col0 + ncols
                ps = psum.tile([P, ncols], F32, name="ps")
                nc.tensor.matmul(ps, tri_t, e_t[:, lo:hi], start=True, stop=False)
                nc.tensor.matmul(
                    ps, onesq_t, esum[j % 2][:, lo:hi], start=False, stop=True
                )
                if not last:
                    nc.vector.tensor_add(
                        esum[(j + 1) % 2][:, lo:hi], esum[j % 2][:, lo:hi], e_t[:, lo:hi]
                    )
                rec = ep.tile([P, ncols], F32, name="rec")
                nc.vector.reciprocal(rec, ps)
                c0 = j * BC + col0
                nc.vector.tensor_mul(sel[:, c0 : c0 + ncols], e_t[:, lo:hi], rec)

            def process_block_pieces(j, hst, last=False):
                """Process block j in two 4-col pieces (ramp/tail blocks)."""
                half = BC // 2
                for piece in range(2):
                    lo = piece * half
                    prod = prodp.tile([P, half, H], F16, name="prod")
                    nc.vector.tensor_tensor(
                        prod, hst[:, lo : lo + half, :], wrep_v[:, lo : lo + half, :],
                        op=mybir.AluOpType.mult,
                    )
                    s_col = scp.tile([P, half], F32, name="s_col")
                    e_t = ep.tile([P, half], F32, name="e_t")
                    reduce_cols(prod, s_col, 0, half, act_cols=2)
                    nc.scalar.activation(e_t, s_col, Exp)
                    # e_t covers columns [lo, lo+half) of block j
                    ps = psum.tile([P, half], F32, name="ps")
                    nc.tensor.matmul(ps, tri_t, e_t, start=True, stop=False)
                    nc.tensor.matmul(
                        ps, onesq_t, esum[j % 2][:, lo : lo + half],
                        start=False, stop=True,
                    )
                    if not last:
                        nc.vector.tensor_add(
                            esum[(j + 1) % 2][:, lo : lo + half],
                            esum[j % 2][:, lo : lo + half],
                            e_t,
                        )
                    rec = ep.tile([P, half], F32, name="rec")
                    nc.vector.reciprocal(rec, ps)
                    c0 = j * BC + lo
                    nc.vector.tensor_mul(sel[:, c0 : c0 + half], e_t, rec)

            def process_block(j, hst, act_cols):
                """Whole-block processing (own mult)."""
                prod = prodp.tile([P, BC, H], F16, name="prod")
                nc.vector.tensor_tensor(prod, hst, wrep_v, op=mybir.AluOpType.mult)
                s_col = scp.tile([P, BC], F32, name="s_col")
                reduce_cols(prod, s_col, 0, BC, act_cols)
                e_t = ep.tile([P, BC], F32, name="e_t")
                nc.scalar.activation(e_t, s_col, Exp)
                scan_block(j, e_t, BC)

            def process_pair(jp, pair, act_cols_a, act_cols_b):
                """Pair processing: one mult for both blocks, then per-block
                reduction/scan.  Sub-index 1 = block jp, 0 = block jp+1."""
                prod = prodp.tile([P, 2, BC, H], F16, name="prod")
                pv = pair.rearrange("p two b h -> p (two b h)")
                nc.vector.tensor_tensor(
                    prod.rearrange("p two b h -> p (two b h)"),
                    pv,
                    wrep2_v,
                    op=mybir.AluOpType.mult,
                )
                for j, sub, act_cols in (
                    (jp, 1, act_cols_a),
                    (jp + 1, 0, act_cols_b),
                ):
                    s_col = scp.tile([P, BC], F32, name="s_col")
                    reduce_cols(prod[:, sub], s_col, 0, BC, act_cols)
                    e_t = ep.tile([P, BC], F32, name="e_t")
                    nc.scalar.activation(e_t, s_col, Exp)
                    scan_block(j, e_t, BC)

            wrep2_t = consts.tile([P, 2 * BC * H], F16)
            nc.scalar.dma_start(
                out=wrep2_t.rearrange("p (two c) -> p two c", two=2)[:, 0],
                in_=wrep.ap(),
            )
            nc.scalar.dma_start(
                out=wrep2_t.rearrange("p (two c) -> p two c", two=2)[:, 1],
                in_=wrep.ap(),
            )
            wrep2_v = wrep2_t

            process_block_pieces(0, dmas[0])
            process_block(1, dmas[1], act_cols=5)
            for jp in range(2, NBLK - 2, 2):
                process_pair(jp, dmas[(jp, "pair")], act_cols_a=5, act_cols_b=4)
            process_block(30, dmas[30], act_cols=4)
            process_block_pieces(31, dmas[31], last=True)

            # out chunks: [0,16), [16,24), [24,30), [30,32) blocks
            for glo, ghi in ((0, 16), (16, 24), (24, 30), (30, 32)):
                lo, hi = glo * BC, ghi * BC
                nc.sync.dma_start(out=out_ap[:, lo:hi], in_=sel[:, lo:hi])

    nc.compile()
    return nc


_PROGRAM = None


def _get_program():
    global _PROGRAM
    if _PROGRAM is None:
        _PROGRAM = build_program()
    return _PROGRAM


def make_in_maps(hidden_states, W):
    hidden_states = np.asarray(hidden_states, dtype=np.float32)
    w16 = np.asarray(W, dtype=np.float32)[:, 0].astype(np.float16)
    wrep = np.ascontiguousarray(np.tile(w16[None, :], (P, BC)))
    tri = np.tril(np.ones((P, P), dtype=np.float32))
    onesq = np.ones((P, P), dtype=np.float32)
    in_maps = []
    for c in range(N_CORES):
        hs_c = np.ascontiguousarray(hidden_states[:, c * BC : (c + 1) * BC, :])
        in_maps.append({"hs": hs_c, "wrep": wrep, "tri": tri, "onesq": onesq})
    return in_maps


def assemble_output(results):
    cols = []
    for c in range(N_CORES):
        oc = results[c]["out"]  # [P, C], col = j*BC + b, j = reversed block
        full = oc.reshape(P, NBLK, BC)[:, ::-1, :].transpose(1, 0, 2).reshape(S, BC)
        cols.append(full)
    return np.concatenate(cols, axis=1)[..., None].astype(np.float32)


def kernel(hidden_states, W, b):
    nc = _get_program()
    in_maps = make_in_maps(hidden_states, W)
    res = bass_utils.run_bass_kernel_spmd(nc, in_maps, core_ids=list(range(N_CORES)))
    return assemble_output(res.results)
